# revision 3
# baseline (speedup 1.0000x reference)
"""DGCNN (2x EdgeConv + global mean pool + MLP) on Trainium2, 8 NeuronCores.

Fully on-device EdgeConv: per core 8 graphs; for each graph the kernel
computes the kNN score matrix on TensorE (augmented matmul folding in
-|x_j|^2), extracts top-10 neighbors with DVE max/max_index/match_replace,
wraps the indices into the GPSIMD ap_gather layout with small DMAs,
gathers neighbor features, and runs the edge MLPs + sum-aggregation +
mean-pool reduction on device. Only the tiny pooled classifier MLP
(192->1024->512->256->3 on 64 graph vectors) runs on host.
"""
import sys

sys.path.insert(0, "/opt/trn_rl_repo")
sys.path.insert(0, "/opt/trn_rl_repo/concourse")

import numpy as np

N_CORES = 8
B = 64
NP = 1024
GPC = B // N_CORES     # graphs per core
K = 10
NE = NP * K            # edges per graph
SLOPE = 0.01

_CACHE = {}


def _lrelu(v):
    return np.where(v >= 0, v, SLOPE * v)


def _build():
    import concourse.mybir as mybir
    from concourse import bacc, library_config
    from concourse.tile import TileContext
    from contextlib import ExitStack

    dt = mybir.dt
    F32 = dt.float32
    I16 = dt.int16
    U16 = dt.uint16
    IDENT = mybir.ActivationFunctionType.Identity
    ADD = mybir.AluOpType.add
    MULT = mybir.AluOpType.mult
    MAX = mybir.AluOpType.max
    AX = mybir.AxisListType.X

    nc = bacc.Bacc("TRN2", target_bir_lowering=False, debug=False,
                   num_devices=N_CORES)

    def din(name, shape, dtype=F32):
        return nc.dram_tensor(name, shape, dtype, kind="ExternalInput").ap()

    xT = din("xT", [4, GPC * NP])
    w1u = din("w1u", [4, 64])
    w1v = din("w1v", [4, 64])
    b1a = din("b1a", [64, 1])
    w1b = din("w1b", [64, 64])
    b1b = din("b1b", [64, 1])
    w1c = din("w1c", [64, 64])
    b1c = din("b1c", [64, 1])
    w2u = din("w2u", [64, 128])
    w2v = din("w2v", [64, 128])
    b2 = din("b2", [128, 1])
    out1 = nc.dram_tensor("out1", [64, GPC], F32, kind="ExternalOutput").ap()
    out2 = nc.dram_tensor("out2", [128, GPC], F32, kind="ExternalOutput").ap()

    with TileContext(nc) as tc:
        ctx = ExitStack()
        cst = ctx.enter_context(tc.tile_pool(name="cst", bufs=1))
        wk = ctx.enter_context(tc.tile_pool(name="wk", bufs=1))
        pss = ctx.enter_context(tc.tile_pool(name="pss", bufs=2, space="PSUM"))

        nc.gpsimd.load_library(library_config.ap_gather)

        def load(ap_in, shape, dtype=F32):
            t = cst.tile(shape, dtype, tag=ap_in.name)
            nc.sync.dma_start(out=t, in_=ap_in)
            return t

        w1u_s = load(w1u, [4, 64])
        w1v_s = load(w1v, [4, 64])
        b1a_s = load(b1a, [64, 1])
        w1b_s = load(w1b, [64, 64])
        b1b_s = load(b1b, [64, 1])
        w1c_s = load(w1c, [64, 64])
        b1c_s = load(b1c, [64, 1])
        w2u_s = load(w2u, [64, 128])
        w2v_s = load(w2v, [64, 128])
        b2_s = load(b2, [128, 1])

        ones1 = cst.tile([1, 128], F32, tag="ones1")
        nc.vector.memset(ones1, 1.0)
        ones64 = cst.tile([64, 1], F32, tag="ones64")
        nc.vector.memset(ones64, 1.0)

        pooled1 = cst.tile([64, GPC], F32, tag="pooled1")
        pooled2 = cst.tile([128, GPC], F32, tag="pooled2")

        # big shared work tiles (40 KB/partition each)
        G_vg = cst.tile([128, NE], F32, tag="G_vg")
        G_h = cst.tile([128, NE], F32, tag="G_h")
        G_hl = cst.tile([128, NE], F32, tag="G_hl")

        def knn_wrap(feat, featx2, nsq, wfull, n_groups):
            """feat/featx2: [P, 1024]; nsq: [1, 1024] (= -|x_j|^2).
            Fills wfull [16*n_groups, 640] int16 with wrapped top-10 idx;
            edge order e = 1280*b + 160*q + 16*k + r, node i = 128*b+16*q+r.
            """
            for b in range(8):
                S = wk.tile([128, NP], F32, tag="S", bufs=2)
                for h in range(2):
                    ps = pss.tile([128, 512], F32, tag="ps", bufs=4)
                    nc.tensor.matmul(ps, featx2[:, 128 * b:128 * (b + 1)],
                                     feat[:, 512 * h:512 * (h + 1)],
                                     start=True, stop=False)
                    nc.tensor.matmul(ps, ones1, nsq[:, 512 * h:512 * (h + 1)],
                                     start=False, stop=True)
                    nc.scalar.activation(S[:, 512 * h:512 * (h + 1)], ps, IDENT)
                v8a = wk.tile([128, 8], F32, tag="v8a", bufs=2)
                i10 = wk.tile([128, 16], U16, tag="i10", bufs=2)
                nc.vector.max(v8a, S)
                nc.vector.max_index(i10[:, 0:8], v8a, S)
                S2 = wk.tile([128, NP], F32, tag="S2", bufs=2)
                nc.vector.match_replace(S2, v8a, S, -1e30)
                v8b = wk.tile([128, 8], F32, tag="v8b", bufs=2)
                nc.vector.max(v8b, S2)
                nc.vector.max_index(i10[:, 8:16], v8b, S2)
                for q in range(8):
                    nc.sync.dma_start(
                        out=wfull[0:16, 80 * b + 10 * q:80 * b + 10 * q + 10],
                        in_=i10[16 * q:16 * (q + 1), 0:10].bitcast(I16))
            for grp in range(1, n_groups):
                nc.sync.dma_start(out=wfull[16 * grp:16 * (grp + 1), :],
                                  in_=wfull[0:16, :])

        def uv(feat, wu, wv, bias, P):
            """u = feat^T @ wu + bias, v = feat^T @ wv; returns uT,vT [P,1024]."""
            uT = wk.tile([P, NP], F32, tag="uT")
            vT = wk.tile([P, NP], F32, tag="vT")
            for h in range(2):
                pu = pss.tile([P, 512], F32, tag="ps", bufs=4)
                nc.tensor.matmul(pu, wu, feat[:, 512 * h:512 * (h + 1)],
                                 start=True, stop=True)
                nc.scalar.activation(uT[:, 512 * h:512 * (h + 1)], pu, IDENT,
                                     bias=bias)
                pv = pss.tile([P, 512], F32, tag="ps", bufs=4)
                nc.tensor.matmul(pv, wv, feat[:, 512 * h:512 * (h + 1)],
                                 start=True, stop=True)
                nc.scalar.activation(vT[:, 512 * h:512 * (h + 1)], pv, IDENT)
            return uT, vT

        def bcast_add_lrelu(uT, P):
            """G_h[:P] = G_vg[:P] + u_bcast ; G_hl[:P] = lrelu(G_h[:P])."""
            for b in range(8):
                sl = slice(1280 * b, 1280 * (b + 1))
                vg = G_vg[0:P, sl].rearrange("c (q k r) -> c q k r",
                                             q=8, k=K, r=16)
                ub = uT[:, 128 * b:128 * (b + 1)].rearrange(
                    "c (q r) -> c q r", q=8).unsqueeze(2).broadcast_to(
                    [P, 8, K, 16])
                ho = G_h[0:P, sl].rearrange("c (q k r) -> c q k r",
                                            q=8, k=K, r=16)
                nc.vector.tensor_tensor(out=ho, in0=vg, in1=ub, op=ADD)
            # exact lrelu: max(z, 0.01*z)
            nc.vector.scalar_tensor_tensor(
                out=G_hl[0:P, :], in0=G_h[0:P, :], scalar=SLOPE,
                in1=G_h[0:P, :], op0=MULT, op1=MAX)

        def ksum(src, P, out_ap):
            """out[c, i] = sum_k src[c, e(i,k)] for the wrap edge order."""
            red = src.rearrange("c (bq k r) -> c bq r k", bq=64, k=K, r=16)
            nc.vector.tensor_reduce(out=out_ap.rearrange(
                "c (bq r) -> c bq r", r=16), in_=red, axis=AX, op=ADD)

        for g in range(GPC):
            xg = wk.tile([4, NP], F32, tag="xg")
            nc.sync.dma_start(out=xg, in_=xT[:, NP * g:NP * (g + 1)])

            # ---------------- conv1 (d=4) ----------------
            xg2 = wk.tile([4, NP], F32, tag="fx2")
            nc.vector.tensor_scalar_mul(xg2, xg, 2.0)
            xsq = wk.tile([4, NP], F32, tag="fsq")
            nc.vector.tensor_tensor(out=xsq, in0=xg, in1=xg, op=MULT)
            nsq1 = wk.tile([1, NP], F32, tag="nsq")
            for h in range(2):
                pq = pss.tile([1, 512], F32, tag="pq", bufs=2)
                nc.tensor.matmul(pq, ones64[0:4, :],
                                 xsq[:, 512 * h:512 * (h + 1)],
                                 start=True, stop=True)
                nc.scalar.activation(nsq1[:, 512 * h:512 * (h + 1)], pq,
                                     IDENT, scale=-1.0)
            wc1 = wk.tile([64, 640], I16, tag="wc")
            knn_wrap(xg, xg2, nsq1, wc1, 4)

            u1T, v1T = uv(xg, w1u_s, w1v_s, b1a_s, 64)
            nc.gpsimd.ap_gather(G_vg[0:64, :], v1T, wc1, channels=64,
                                num_elems=NP, d=1, num_idxs=NE)
            bcast_add_lrelu(u1T, 64)

            # edge MLP: two 64->64 layers on [64, 10240]
            for c in range(NE // 512):
                pm = pss.tile([64, 512], F32, tag="ps", bufs=4)
                nc.tensor.matmul(pm, w1b_s, G_hl[0:64, 512 * c:512 * (c + 1)],
                                 start=True, stop=True)
                tz = wk.tile([64, 512], F32, tag="tz", bufs=4)
                nc.scalar.activation(tz, pm, IDENT, bias=b1b_s)
                nc.vector.scalar_tensor_tensor(
                    out=G_h[0:64, 512 * c:512 * (c + 1)], in0=tz, scalar=SLOPE,
                    in1=tz, op0=MULT, op1=MAX)
            for c in range(NE // 512):
                pm = pss.tile([64, 512], F32, tag="ps", bufs=4)
                nc.tensor.matmul(pm, w1c_s, G_h[0:64, 512 * c:512 * (c + 1)],
                                 start=True, stop=True)
                tz = wk.tile([64, 512], F32, tag="tz", bufs=4)
                nc.scalar.activation(tz, pm, IDENT, bias=b1c_s)
                nc.vector.scalar_tensor_tensor(
                    out=G_hl[0:64, 512 * c:512 * (c + 1)], in0=tz, scalar=SLOPE,
                    in1=tz, op0=MULT, op1=MAX)

            x1g = wk.tile([64, NP], F32, tag="x1g")
            ksum(G_hl[0:64, :], 64, x1g)
            nc.vector.tensor_reduce(out=pooled1[:, g:g + 1], in_=x1g,
                                    axis=AX, op=ADD)

            # ---------------- conv2 (d=64) ----------------
            x1g2 = wk.tile([64, NP], F32, tag="fx2")
            nc.vector.tensor_scalar_mul(x1g2, x1g, 2.0)
            x1sq = wk.tile([64, NP], F32, tag="fsq")
            nc.vector.tensor_tensor(out=x1sq, in0=x1g, in1=x1g, op=MULT)
            nsq2 = wk.tile([1, NP], F32, tag="nsq")
            for h in range(2):
                pq = pss.tile([1, 512], F32, tag="pq", bufs=2)
                nc.tensor.matmul(pq, ones64, x1sq[:, 512 * h:512 * (h + 1)],
                                 start=True, stop=True)
                nc.scalar.activation(nsq2[:, 512 * h:512 * (h + 1)], pq,
                                     IDENT, scale=-1.0)
            wc2 = wk.tile([128, 640], I16, tag="wc")
            knn_wrap(x1g, x1g2, nsq2, wc2, 8)

            u2T, v2T = uv(x1g, w2u_s, w2v_s, b2_s, 128)
            nc.gpsimd.ap_gather(G_vg, v2T, wc2, channels=128,
                                num_elems=NP, d=1, num_idxs=NE)
            bcast_add_lrelu(u2T, 128)

            x2g = wk.tile([128, NP], F32, tag="x2g")
            ksum(G_hl, 128, x2g)
            nc.vector.tensor_reduce(out=pooled2[:, g:g + 1], in_=x2g,
                                    axis=AX, op=ADD)

        nc.sync.dma_start(out=out1, in_=pooled1)
        nc.sync.dma_start(out=out2, in_=pooled2)
        ctx.close()

    nc.compile()
    return nc


def _prep_inputs(x, pos, w1a, b1a, w1b, b1b, w1c, b1c, w2, b2):
    f32 = np.float32
    xx = np.concatenate([np.asarray(x, f32), np.asarray(pos, f32)], 1)
    xx = xx.reshape(B, NP, 4)
    w1a = np.asarray(w1a, f32)
    w2 = np.asarray(w2, f32)
    common = {
        "w1u": np.ascontiguousarray(w1a[:4] - w1a[4:]),
        "w1v": np.ascontiguousarray(w1a[4:]),
        "b1a": np.asarray(b1a, f32).reshape(64, 1),
        "w1b": np.ascontiguousarray(np.asarray(w1b, f32)),
        "b1b": np.asarray(b1b, f32).reshape(64, 1),
        "w1c": np.ascontiguousarray(np.asarray(w1c, f32)),
        "b1c": np.asarray(b1c, f32).reshape(64, 1),
        "w2u": np.ascontiguousarray(w2[:64] - w2[64:]),
        "w2v": np.ascontiguousarray(w2[64:]),
        "b2": np.asarray(b2, f32).reshape(128, 1),
    }
    in_maps = []
    for c in range(N_CORES):
        m = dict(common)
        m["xT"] = np.ascontiguousarray(
            xx[c * GPC:(c + 1) * GPC].transpose(2, 0, 1).reshape(4, GPC * NP))
        in_maps.append(m)
    return in_maps


def kernel(x, pos, batch, w1a, b1a, w1b, b1b, w1c, b1c, w2, b2,
           wl, bl, wm1, bm1, wm2, bm2, wm3, bm3):
    from concourse.bass_utils import run_bass_kernel_spmd

    f32 = np.float32
    in_maps = _prep_inputs(x, pos, w1a, b1a, w1b, b1b, w1c, b1c, w2, b2)

    if "nc" not in _CACHE:
        _CACHE["nc"] = _build()
    res = run_bass_kernel_spmd(_CACHE["nc"], in_maps, list(range(N_CORES)))

    pooled = np.concatenate(
        [np.concatenate([res.results[c]["out1"], res.results[c]["out2"]],
                        axis=0).T for c in range(N_CORES)], axis=0)

    # host classifier on pooled means [64, 192]
    P = pooled.astype(f32) / NP
    o = P @ np.asarray(wl, f32) + np.asarray(bl, f32)
    o = _lrelu(o @ np.asarray(wm1, f32) + np.asarray(bm1, f32))
    o = _lrelu(o @ np.asarray(wm2, f32) + np.asarray(bm2, f32))
    o = o @ np.asarray(wm3, f32) + np.asarray(bm3, f32)
    return o.astype(f32)
